# revision 11
# baseline (speedup 1.0000x reference)
"""CosineAttention Trainium2 Bass kernel.

Computes softmax(cos_sim(keys[b,l,:], query[b,:]) masked) over l, for
B=64, L=4096, D=1024, sharded batch-parallel over 8 NeuronCores
(8 batches per core, 128 MiB of keys per core -> memory bound).

Math per (b, l):
    dot[l]  = sum_d keys[b,l,d] * qhat[b,d]      (qhat = q / max(||q||, eps))
    ssq[l]  = sum_d keys[b,l,d]^2
    score   = dot / max(sqrt(ssq), eps) + (mask-1)*1e30
    out     = exp(score) / sum_l exp(score)      (scores in [-1,1]: no max-sub needed)

Engine plan per core:
  - DMA   : stream keys in 4 MiB chunks (contiguous 32 KiB per partition)
  - DVE   : fused tensor_tensor_reduce (mult + add-reduce) -> dot   (1 pass)
  - ACT   : fused activation(Square, accum_out=)           -> ssq   (1 pass)
  - PE    : ones-matmul for the cross-partition softmax denominator
L is laid out interleaved: l = p*T + t (p = partition, T = L/128), so both
the keys loads and the output store are contiguous per partition.
"""

import numpy as np

import concourse.bass as bass
import concourse.tile as tile
from concourse import bacc, mybir

P = 128          # SBUF partitions
B = 64           # full batch
L = 4096
D = 1024
N_CORES = 8
BPC = B // N_CORES   # batches per core
CJ = 8               # l-tiles per DMA chunk (4 MiB chunks)

F32 = mybir.dt.float32
U8 = mybir.dt.uint8
Alu = mybir.AluOpType
Act = mybir.ActivationFunctionType

EPS = 1e-12
NEG_BIG = 1.0e30


def build_nc(bpc=BPC, l_dim=L, d=D, cj=CJ, n_cores=N_CORES, reps=1):
    t_cols = l_dim // P       # score columns per partition
    nch = t_cols // cj        # chunks per batch
    assert t_cols * P == l_dim and nch * cj == t_cols

    nc = bacc.Bacc(
        "TRN2",
        target_bir_lowering=False,
        debug=False,
        enable_asserts=False,
        num_devices=n_cores,
    )

    q_t = nc.dram_tensor("q", [bpc, d], F32, kind="ExternalInput")
    keys_t = nc.dram_tensor("keys", [bpc, l_dim, d], F32, kind="ExternalInput")
    mask_t = nc.dram_tensor("mask", [bpc, l_dim], U8, kind="ExternalInput")
    out_t = nc.dram_tensor("out", [bpc, l_dim], F32, kind="ExternalOutput")

    q_ap = q_t.ap()
    keys_ap = keys_t.ap()
    mask_ap = mask_t.ap()
    out_ap = out_t.ap()

    with tile.TileContext(nc) as tc:
        with (
            tc.tile_pool(name="kpool", bufs=3) as kpool,
            tc.tile_pool(name="singles", bufs=1) as singles,
            tc.tile_pool(name="ascr", bufs=2) as ascr,
            tc.tile_pool(name="psum", bufs=1, space="PSUM") as psum,
        ):
            # --- persistent tiles ---
            qrep = singles.tile([P, bpc, d], F32)        # q replicated to all partitions
            maskf = singles.tile([P, bpc * t_cols], F32) # mask -> additive bias
            qss = singles.tile([P, bpc], F32)            # per-batch ||q||^2
            ones = singles.tile([P, P], F32)             # for cross-partition sum matmul
            negbig = singles.tile([P, 1], F32)           # bias tile for mask rescale

            vdummy = singles.tile([P, 1], F32)           # step-0 sink for fused dot

            nc.vector.memset(ones, 1.0)
            nc.vector.memset(negbig, -NEG_BIG)

            # Broadcast q to all 128 partitions during the DMA (partition step 0).
            q_bcast = bass.AP(
                tensor=q_ap.tensor,
                offset=q_ap.offset,
                ap=[[0, P], [d, bpc], [1, d]],
            )
            nc.gpsimd.dma_start(out=qrep, in_=q_bcast)

            # Mask: u8 -> f32 cast during DMA.  DRAM layout per batch is
            # [P, t_cols] with l = p*t_cols + t.
            mask_v = mask_ap.rearrange("b (p t) -> p b t", p=P)
            nc.gpsimd.dma_start(
                out=maskf[:].rearrange("p (b t) -> p b t", b=bpc), in_=mask_v
            )

            # mask -> additive bias {0, -1e30}, done once up front
            nc.scalar.activation(out=maskf, in_=maskf, func=Act.Identity,
                                 bias=negbig[:, 0:1], scale=NEG_BIG)

            # --- normalize q in place (per-partition identical values) ---
            for b in range(bpc):
                s = ascr.tile([P, d], F32)
                nc.scalar.activation(out=s, in_=qrep[:, b, :], func=Act.Square,
                                     accum_out=qss[:, b : b + 1])
            nc.scalar.activation(out=qss, in_=qss, func=Act.Sqrt)
            nc.vector.tensor_scalar_max(qss, qss, EPS)
            nc.vector.reciprocal(qss, qss)
            for b in range(bpc):
                nc.vector.tensor_scalar_mul(qrep[:, b, :], qrep[:, b, :],
                                            qss[:, b : b + 1])

            for _rep in range(reps):
                # per-rep accumulators (bufs=1 tags -> reps serialize on slots)
                dots = singles.tile([P, bpc * t_cols], F32, tag="dots")
                ssqs = singles.tile([P, bpc * t_cols], F32, tag="ssqs")

                # --- main loop: stream keys, fused dot + ssq reductions ---
                for b in range(bpc):
                    kv = keys_ap[b].rearrange("(p c j) d -> p c (j d)", p=P, c=nch)
                    for c in range(nch):
                        kt = kpool.tile([P, cj, d], F32, tag="kt")
                        nc.sync.dma_start(out=kt[:].rearrange("p c d -> p (c d)"),
                                          in_=kv[:, c, :])
                        for j in range(cj):
                            idx = b * t_cols + c * cj + j
                            nc.vector.scalar_tensor_tensor(
                                out=vdummy.broadcast_to((P, d)),
                                in0=kt[:, j, :],
                                scalar=1.0,
                                in1=qrep[:, b, :],
                                op0=Alu.mult,
                                op1=Alu.mult,
                                accum_out=dots[:, idx : idx + 1],
                            )
                            aout = ascr.tile([P, d], F32, tag="aout")
                            nc.scalar.activation(
                                out=aout,
                                in_=kt[:, j, :],
                                func=Act.Square,
                                accum_out=ssqs[:, idx : idx + 1],
                            )

                # --- epilogue (all tiny [P, bpc*t_cols] ops) ---
                nc.scalar.activation(out=ssqs, in_=ssqs, func=Act.Sqrt)   # ||k||
                nc.vector.tensor_scalar_max(ssqs, ssqs, EPS)
                nc.vector.reciprocal(ssqs, ssqs)                          # 1/||k||
                nc.vector.tensor_mul(dots, dots, ssqs)                    # cosine
                nc.vector.tensor_add(dots, dots, maskf)
                nc.scalar.activation(out=dots, in_=dots, func=Act.Exp)

                # softmax denominator: ones.T @ E sums across partitions;
                # then reduce each batch's t_cols columns; every partition
                # ends up with the full sum for each batch.
                mm = psum.tile([P, bpc * t_cols], F32, tag="mm")
                nc.tensor.matmul(out=mm, lhsT=ones, rhs=dots, start=True,
                                 stop=True)
                den = singles.tile([P, bpc], F32, tag="den")
                nc.vector.tensor_reduce(
                    out=den,
                    in_=mm[:].rearrange("p (b t) -> p b t", b=bpc),
                    axis=mybir.AxisListType.X,
                    op=Alu.add,
                )
                nc.vector.reciprocal(den, den)
                for b in range(bpc):
                    nc.vector.tensor_scalar_mul(
                        dots[:, b * t_cols : (b + 1) * t_cols],
                        dots[:, b * t_cols : (b + 1) * t_cols],
                        den[:, b : b + 1],
                    )

                out_v = out_ap.rearrange("b (p t) -> p b t", p=P)
                nc.sync.dma_start(
                    out=out_v, in_=dots[:].rearrange("p (b t) -> p b t", b=bpc)
                )

    nc.compile()
    return nc


_NC_CACHE = None


def _get_nc():
    global _NC_CACHE
    if _NC_CACHE is None:
        _NC_CACHE = build_nc()
    return _NC_CACHE


def kernel(query: np.ndarray, keys: np.ndarray, mask: np.ndarray) -> np.ndarray:
    assert query.shape == (B, D) and keys.shape == (B, L, D) and mask.shape == (B, L)
    from concourse.bass_utils import run_bass_kernel_spmd

    nc = _get_nc()
    mask_u8 = np.ascontiguousarray(mask).view(np.uint8)
    in_maps = []
    for i in range(N_CORES):
        sl = slice(i * BPC, (i + 1) * BPC)
        in_maps.append(
            {
                "q": np.ascontiguousarray(query[sl], dtype=np.float32),
                "keys": np.ascontiguousarray(keys[sl], dtype=np.float32),
                "mask": np.ascontiguousarray(mask_u8[sl]),
            }
        )
    res = run_bass_kernel_spmd(nc, in_maps, core_ids=list(range(N_CORES)))
    out = np.concatenate([r["out"] for r in res.results], axis=0)
    return out.astype(np.float32, copy=False)
